# revision 3
# baseline (speedup 1.0000x reference)
"""Multi-head causal attention (B=2, S=2048, D=1024, H=16) on 8 trn2 cores.

Sharding: DP on batch (2 groups of 4 cores), TP on heads within a group
(4 heads/core, column-parallel QKV). The output projection is sharded by
OUTPUT column: per-core attn^T slices are AllGather'd (bf16, on the c
axis) within each 4-core group and each core computes
out[:, r*256:(r+1)*256] = attn_full @ Wo[:, slice] + bo[slice].
Host does pure slicing/concat only.

Per-core pipeline (bf16 matmuls, fp32 PSUM accumulation):
  A. x column-slices cast fp32->bf16 via SWDGE (contiguous-dest), then
     contiguous-source xbar DMA transposes -> x^T SBUF [128c, 1024s].
  B. q^T/k^T = W.T @ x; bias added on DVE (per-partition tensor_scalar),
     bf16. v natural [s, hd] with per-head 65-stride layout + ones
     column (softmax denominator rides the PV matmul).
  C. flash-style per 512-wide sq chunk: scores^T for a head PAIR land
     in one 2-bank PSUM tile (K=64 row-group packing); ONE wide exp per
     pair on ACT (scale=1/8, no max subtraction); causal masking via
     gpsimd memset+affine_select; PV accumulates immediately.
  D. software-pipelined: AllGather of attn^T issued right after each
     chunk; the chunk's output projection is emitted inside the NEXT
     chunk's compute so the collective is fully hidden.
"""

import sys

sys.path.insert(0, "/opt/trn_rl_repo")

from contextlib import ExitStack

import numpy as np

import concourse.bacc as bacc
import concourse.bass as bass
import concourse.tile as tile
from bass_rust import add_dep_helper
from concourse import mybir
from concourse.bass_utils import run_bass_kernel_spmd
from concourse.masks import make_identity

F32 = mybir.dt.float32
BF16 = mybir.dt.bfloat16
EXPF = mybir.ActivationFunctionType.Exp

B, S, D, H, HD = 2, 2048, 1024, 16, 64
TP = 4  # cores per batch group
HPC = H // TP  # 4 local heads
DH = HPC * HD  # 256 local head dims
NK = D // 128  # 8 contraction tiles
NST = S // 128  # 16 s tiles
NSC = S // 512  # 4 s chunks
SCALE = 1.0 / 8.0  # 1/sqrt(HD)

REPLICA_GROUPS = [[0, 1, 2, 3], [4, 5, 6, 7]]


def build_nc():
    nc = bacc.Bacc("TRN2", target_bir_lowering=False, debug=False, num_devices=8)
    X = nc.dram_tensor("x", [S, D], F32, kind="ExternalInput")
    WQ = nc.dram_tensor("wq", [D, DH], F32, kind="ExternalInput")
    WK = nc.dram_tensor("wk", [D, DH], F32, kind="ExternalInput")
    WV = nc.dram_tensor("wv", [D, DH], F32, kind="ExternalInput")
    WO = nc.dram_tensor("wo", [D, DH], F32, kind="ExternalInput")
    BQ = nc.dram_tensor("bq2", [128, 2], F32, kind="ExternalInput")
    BK = nc.dram_tensor("bk2", [128, 2], F32, kind="ExternalInput")
    BV = nc.dram_tensor("bv_row", [1, DH], F32, kind="ExternalInput")
    BO = nc.dram_tensor("bo_row", [1, DH], F32, kind="ExternalInput")
    OUT = nc.dram_tensor("out", [S, DH], F32, kind="ExternalOutput")

    with tile.TileContext(nc) as tc, ExitStack() as ctx:
        xtp = ctx.enter_context(tc.tile_pool(name="xtp", bufs=1))
        ptp = ctx.enter_context(tc.tile_pool(name="ptp", bufs=1))
        qkv = ctx.enter_context(tc.tile_pool(name="qkv", bufs=1))
        wp = ctx.enter_context(tc.tile_pool(name="wp", bufs=1))
        attp = ctx.enter_context(tc.tile_pool(name="attp", bufs=1))
        atfp = ctx.enter_context(tc.tile_pool(name="atfp", bufs=1))
        outp = ctx.enter_context(tc.tile_pool(name="outp", bufs=1))
        misc = ctx.enter_context(tc.tile_pool(name="misc", bufs=1))
        psum = ctx.enter_context(tc.tile_pool(name="psum", bufs=1, space="PSUM"))
        dram = ctx.enter_context(tc.tile_pool(name="dram", bufs=1, space="DRAM"))

        def ps_big(name):  # 2-bank PSUM slots (scores / proj / tr / po)
            return psum.tile([128, 1024], F32, tag="ps_sc", bufs=2, name=name)

        # ---------- stage A: x^T. Cast column-slices of x (contiguous
        # dest) so the xbar transposes read CONTIGUOUS DRAM at full
        # rate. Traced first; order-only fences keep other DMAs out
        # from between the transposes (HW wait budget 1). ----------
        xt = [[None, None] for _ in range(NK)]  # xt[k][h2]: [128c, 1024s]
        last_xp = None
        for h2 in range(2):
            for k in range(NK):
                xc = dram.tile(
                    [1024, 128], BF16, tag=f"xc{h2}_{k}", name=f"xc{h2}_{k}"
                )
                cast = nc.gpsimd.dma_start(
                    xc[:],
                    X[1024 * h2 : 1024 * (h2 + 1), 128 * k : 128 * (k + 1)],
                )
                if last_xp is not None:
                    add_dep_helper(
                        cast.ins, last_xp.ins, sync=False, reason="cast after xp"
                    )
                t = xtp.tile([128, 1024], BF16, tag="xt", bufs=16, name=f"xt{k}_{h2}")
                xt[k][h2] = t
                last_xp = nc.sync.dma_start(t[:], xc[:], transpose=True)
        fence = last_xp

        def fence_dma(instr):
            add_dep_helper(
                instr.ins, fence.ins, sync=False, reason="DMA after transposes"
            )
            return instr

        # ---------- constants ----------
        ones_bf = misc.tile([1, 128], BF16, tag="ones", name="ones_bf")
        nc.gpsimd.memset(ones_bf[:], 1.0)
        ident = misc.tile([128, 128], BF16, tag="ident", name="ident")
        make_identity(nc, ident[:])
        bq_sb = misc.tile([128, 2], F32, tag="bq", name="bq_sb")
        fence_dma(nc.gpsimd.dma_start(bq_sb[:], BQ[:]))
        bk_sb = misc.tile([128, 2], F32, tag="bk", name="bk_sb")
        fence_dma(nc.gpsimd.dma_start(bk_sb[:], BK[:]))
        bv_sb = misc.tile([1, DH], BF16, tag="bv", name="bv_sb")
        fence_dma(nc.gpsimd.dma_start(bv_sb[:], BV[:]))  # SWDGE cast f32->bf16
        bo_sb = misc.tile([1, DH], BF16, tag="bo", name="bo_sb")
        fence_dma(nc.gpsimd.dma_start(bo_sb[:], BO[:]))

        # ---------- weights: one cast DMA per matrix, [128, k, 256] ----------
        def load_w(dram_t, tag):
            t = wp.tile([128, NK, DH], BF16, tag=tag, name=tag)
            fence_dma(
                nc.gpsimd.dma_start(
                    t[:], dram_t[:].rearrange("(k p) n -> p k n", p=128)
                )
            )
            return t

        wq_sb = load_w(WQ, "wq_sb")
        wk_sb = load_w(WK, "wk_sb")
        wv_sb = load_w(WV, "wv_sb")
        wo_sb = load_w(WO, "wo_sb")

        # ---------- stage B: projections ----------
        qt, kt = [], []
        for w_sb, b_sb, dst in ((wq_sb, bq_sb, qt), (wk_sb, bk_sb, kt)):
            for m in range(2):
                slab = qkv.tile([128, S], BF16, tag="qkt", bufs=4, name=f"qkt{m}")
                dst.append(slab)
                for h2 in range(2):
                    ps = ps_big("ps_qk")
                    for half in range(2):
                        for k in range(NK):
                            nc.tensor.matmul(
                                ps[:, 512 * half : 512 * (half + 1)],
                                w_sb[:, k, 128 * m : 128 * (m + 1)],
                                xt[k][h2][:, 512 * half : 512 * (half + 1)],
                                start=(k == 0),
                                stop=(k == NK - 1),
                            )
                    nc.vector.tensor_scalar_add(
                        slab[:, 1024 * h2 : 1024 * (h2 + 1)],
                        ps[:],
                        b_sb[:, m : m + 1],
                    )

        # v natural [s, hd], per-head-65-stride layout with ones column
        vt = []
        for i in range(NST):
            h2, o = divmod(i, 8)
            vtile = qkv.tile([128, 4 * 65], BF16, tag="vt", bufs=NST, name=f"vt{i}")
            vt.append(vtile)
            ps = ps_big("ps_v")
            for k in range(NK):
                nc.tensor.matmul(
                    ps[:, 0:DH],
                    xt[k][h2][:, 128 * o : 128 * (o + 1)],
                    wv_sb[:, k, :],
                    start=(k == 0),
                    stop=False,
                )
            nc.tensor.matmul(
                ps[:, 0:DH], ones_bf[0:1, :], bv_sb[0:1, :], start=False, stop=True
            )
            v_dst = vtile[:].rearrange("p (h x) -> p h x", x=65)[:, :, 0:64]
            v_src = ps[:, 0:DH].rearrange("p (h x) -> p h x", x=64)
            nc.vector.tensor_copy(v_dst, v_src)
            one_cols = vtile[:].rearrange("p (h x) -> p h x", x=65)[:, :, 64:65]
            nc.gpsimd.memset(one_cols, 1.0)

        # ---------- stages C+D, flash-style, software-pipelined AG ----------
        attn = [None] * NST
        pending = []  # deferred D2 emitters

        def emit_d2(g, ag_out):
            atf = atfp.tile([128, NK, 512], BF16, tag="atf", bufs=2, name="atf")
            nc.sync.dma_start(atf[:], ag_out[:].rearrange("(k p) n -> p k n", p=128))
            o_sb = outp.tile([128, 4, DH], F32, tag="o_sb", bufs=2, name="o_sb")
            for ii in range(4):
                po = ps_big("ps_out")
                for k in range(NK):
                    nc.tensor.matmul(
                        po[:, 0:DH],
                        atf[:, k, 128 * ii : 128 * (ii + 1)],
                        wo_sb[:, k, :],
                        start=(k == 0),
                        stop=False,
                    )
                nc.tensor.matmul(
                    po[:, 0:DH], ones_bf[0:1, :], bo_sb[0:1, :],
                    start=False, stop=True,
                )
                nc.vector.tensor_copy(o_sb[:, ii, :], po[:, 0:DH])
            nc.sync.dma_start(
                OUT[512 * g : 512 * (g + 1), :].rearrange("(i p) n -> p i n", p=128),
                o_sb[:],
            )

        for g in range(NSC):
            pv = []
            for ii in range(4):
                t = psum.tile(
                    [128, 4 * 65], F32, tag="ps_pv", bufs=4, name=f"pv{ii}"
                )
                pv.append(t)
            for j in range(4 * g + 4):
                d = j - 4 * g  # >= 0 only on diagonal-overlap blocks
                lo = max(0, 128 * d)
                ptile = ptp.tile([128, 4 * 512], BF16, tag="pt", bufs=3, name="pt")
                for hp in range(2):
                    ps = ps_big("ps_sc")
                    for hh in range(2):
                        nc.tensor.matmul(
                            ps[:, 512 * hh : 512 * (hh + 1)],
                            kt[hp][64 * hh : 64 * hh + 64, 128 * j : 128 * (j + 1)],
                            qt[hp][64 * hh : 64 * hh + 64, 512 * g : 512 * (g + 1)],
                            start=True,
                            stop=True,
                        )
                    # one wide exp for the head pair (masked region is
                    # exp'd too -- bounded junk -- then zeroed below)
                    nc.scalar.activation(
                        ptile[:, 1024 * hp : 1024 * (hp + 1)],
                        ps[:],
                        EXPF,
                        bias=0.0,
                        scale=SCALE,
                    )
                    if d >= 0:
                        for hh in range(2):
                            h = 2 * hp + hh
                            if lo > 0:
                                nc.gpsimd.memset(
                                    ptile[:, 512 * h : 512 * h + lo], 0.0
                                )
                            bnd = ptile[:, 512 * h + lo : 512 * h + lo + 128]
                            nc.gpsimd.affine_select(
                                out=bnd,
                                in_=bnd,
                                compare_op=mybir.AluOpType.is_ge,
                                fill=0.0,
                                base=0,
                                pattern=[[1, 128]],
                                channel_multiplier=-1,
                            )
                # PV: consume this p^T block immediately
                for ii in range(max(0, d), 4):
                    i = 4 * g + ii
                    for h in range(4):
                        nc.tensor.matmul(
                            pv[ii][:, 65 * h : 65 * h + 65],
                            ptile[:, 512 * h + 128 * ii : 512 * h + 128 * ii + 128],
                            vt[j][:].rearrange("p (h x) -> p h x", x=65)[:, h, :],
                            start=(j == 0 and h == 0),
                            stop=(j == i and h == 3),
                        )

            # deferred output projection of the previous chunk (its
            # AllGather has been running during this chunk's compute)
            if pending:
                pending.pop(0)()

            # normalize
            for ii in range(4):
                i = 4 * g + ii
                rl = misc.tile([128, 4], F32, tag="rl", bufs=4, name="rl")
                at = attp.tile([128, DH], BF16, tag="attn", bufs=NST, name=f"at{i}")
                attn[i] = at
                lcols = pv[ii][:].rearrange("p (h x) -> p h x", x=65)[:, :, 64:65]
                nc.vector.reciprocal(rl[:].rearrange("p (h x) -> p h x", x=1), lcols)
                for h in range(4):
                    nc.vector.tensor_scalar_mul(
                        at[:, 64 * h : 64 * h + 64],
                        pv[ii][:, 65 * h : 65 * h + 64],
                        rl[:, h : h + 1],
                    )

            # transpose attn chunk, ship to AllGather
            atTs = attp.tile([128, 2 * 512], BF16, tag="atTs", bufs=2, name="atTs")
            for m in range(2):
                for ii in range(4):
                    i = 4 * g + ii
                    tr = psum.tile(
                        [128, 128], BF16, tag="ps_sc", bufs=2, name="ps_tr"
                    )
                    nc.tensor.transpose(
                        tr[:], attn[i][:, 128 * m : 128 * (m + 1)], ident[:]
                    )
                    nc.vector.tensor_copy(
                        atTs[:, 512 * m + 128 * ii : 512 * m + 128 * (ii + 1)],
                        tr[:],
                    )
            ag_in = dram.tile([DH, 512], BF16, tag=f"ag_in{g}", name=f"ag_in{g}")
            nc.sync.dma_start(
                ag_in[:].rearrange("(m p) n -> p m n", p=128),
                atTs[:].rearrange("p (m n) -> p m n", m=2),
            )
            ag_out = dram.tile([D, 512], BF16, tag=f"ag_out{g}", name=f"ag_out{g}")
            nc.gpsimd.collective_compute(
                "AllGather",
                mybir.AluOpType.bypass,
                replica_groups=REPLICA_GROUPS,
                ins=[ag_in.opt()],
                outs=[ag_out.opt()],
            )
            pending.append(lambda g=g, ago=ag_out: emit_d2(g, ago))

        while pending:
            pending.pop(0)()

    nc.compile()
    return nc


_cached = None


def _get_nc():
    global _cached
    if _cached is None:
        _cached = build_nc()
    return _cached


def make_in_maps(x, Wq, bq, Wk, bk, Wv, bv, Wo, bo):
    in_maps = []
    for c in range(8):
        b, r = divmod(c, TP)
        hsl = slice(r * DH, (r + 1) * DH)
        in_maps.append(
            {
                "x": np.ascontiguousarray(x[b]),
                "wq": np.ascontiguousarray(Wq[:, hsl]),
                "wk": np.ascontiguousarray(Wk[:, hsl]),
                "wv": np.ascontiguousarray(Wv[:, hsl]),
                "wo": np.ascontiguousarray(Wo[:, hsl]),
                "bq2": np.ascontiguousarray(bq[hsl].reshape(2, 128).T),
                "bk2": np.ascontiguousarray(bk[hsl].reshape(2, 128).T),
                "bv_row": np.ascontiguousarray(bv[hsl].reshape(1, DH)),
                "bo_row": np.ascontiguousarray(bo[hsl].reshape(1, DH)),
            }
        )
    return in_maps


def assemble(results):
    """results: list of 8 per-core dicts with 'out' [S, DH] f32."""
    full = np.empty((B, S, D), np.float32)
    for b in range(B):
        full[b] = np.concatenate(
            [np.asarray(results[TP * b + r]["out"]) for r in range(TP)], axis=1
        )
    return full


def run(inputs, **kwargs):
    inputs = {k: np.asarray(v) for k, v in inputs.items()}
    nc = _get_nc()
    in_maps = make_in_maps(**inputs)
    return run_bass_kernel_spmd(nc, in_maps, list(range(8)), **kwargs)


def kernel(**inputs):
    return assemble(run(inputs).results)


# revision 4
# speedup vs baseline: 1.1064x; 1.1064x over previous
"""Multi-head causal attention (B=2, S=2048, D=1024, H=16) on 8 trn2 cores.

Sharding: DP on batch (2 groups of 4 cores), TP on heads within a group
(4 heads/core, column-parallel QKV). The output projection is sharded by
OUTPUT column: per-core attn^T slices are AllGather'd (bf16, on the c
axis) within each 4-core group and each core computes
out[:, r*256:(r+1)*256] = attn_full @ Wo[:, slice] + bo[slice].
Host does pure slicing/concat only.

Per-core pipeline (bf16 matmuls, fp32 PSUM accumulation):
  A. x column-slices cast fp32->bf16 via SWDGE (contiguous-dest), then
     contiguous-source xbar DMA transposes -> x^T SBUF [128c, 1024s].
  B. q^T/k^T = W.T @ x; bias added on DVE (per-partition tensor_scalar),
     bf16. v natural [s, hd] with per-head 65-stride layout + ones
     column (softmax denominator rides the PV matmul).
  C. flash-style per 512-wide sq chunk: scores^T for a head PAIR land
     in one 2-bank PSUM tile (K=64 row-group packing); ONE wide exp per
     pair on ACT (scale=1/8, no max subtraction); causal masking via
     gpsimd memset+affine_select; PV accumulates immediately.
  D. software-pipelined: AllGather of attn^T issued right after each
     chunk; the chunk's output projection is emitted inside the NEXT
     chunk's compute so the collective is fully hidden.
"""

import sys

sys.path.insert(0, "/opt/trn_rl_repo")

from contextlib import ExitStack

import numpy as np

import concourse.bacc as bacc
import concourse.bass as bass
import concourse.tile as tile
from bass_rust import add_dep_helper
from concourse import mybir
from concourse.bass_utils import run_bass_kernel_spmd
from concourse.masks import make_identity

F32 = mybir.dt.float32
BF16 = mybir.dt.bfloat16
EXPF = mybir.ActivationFunctionType.Exp

B, S, D, H, HD = 2, 2048, 1024, 16, 64
TP = 4  # cores per batch group
HPC = H // TP  # 4 local heads
DH = HPC * HD  # 256 local head dims
NK = D // 128  # 8 contraction tiles
NST = S // 128  # 16 s tiles
NSC = S // 512  # 4 s chunks
SCALE = 1.0 / 8.0  # 1/sqrt(HD)

REPLICA_GROUPS = [[0, 1, 2, 3], [4, 5, 6, 7]]


def build_nc():
    nc = bacc.Bacc("TRN2", target_bir_lowering=False, debug=False, num_devices=8)
    X = nc.dram_tensor("x", [S, D], F32, kind="ExternalInput")
    WQ = nc.dram_tensor("wq", [D, DH], F32, kind="ExternalInput")
    WK = nc.dram_tensor("wk", [D, DH], F32, kind="ExternalInput")
    WV = nc.dram_tensor("wv", [D, DH], F32, kind="ExternalInput")
    WO = nc.dram_tensor("wo", [D, DH], F32, kind="ExternalInput")
    BQ = nc.dram_tensor("bq2", [128, 2], F32, kind="ExternalInput")
    BK = nc.dram_tensor("bk2", [128, 2], F32, kind="ExternalInput")
    BV = nc.dram_tensor("bv_row", [1, DH], F32, kind="ExternalInput")
    BO = nc.dram_tensor("bo_row", [1, DH], F32, kind="ExternalInput")
    OUT = nc.dram_tensor("out", [S, DH], F32, kind="ExternalOutput")

    with tile.TileContext(nc) as tc, ExitStack() as ctx:
        xtp = ctx.enter_context(tc.tile_pool(name="xtp", bufs=1))
        ptp = ctx.enter_context(tc.tile_pool(name="ptp", bufs=1))
        qkv = ctx.enter_context(tc.tile_pool(name="qkv", bufs=1))
        wp = ctx.enter_context(tc.tile_pool(name="wp", bufs=1))
        attp = ctx.enter_context(tc.tile_pool(name="attp", bufs=1))
        atfp = ctx.enter_context(tc.tile_pool(name="atfp", bufs=1))
        outp = ctx.enter_context(tc.tile_pool(name="outp", bufs=1))
        misc = ctx.enter_context(tc.tile_pool(name="misc", bufs=1))
        psum = ctx.enter_context(tc.tile_pool(name="psum", bufs=1, space="PSUM"))
        dram = ctx.enter_context(tc.tile_pool(name="dram", bufs=1, space="DRAM"))

        def ps_big(name):  # 2-bank PSUM slots (scores / proj / tr / po)
            return psum.tile([128, 1024], F32, tag="ps_sc", bufs=2, name=name)

        # ---------- stage A: x^T. Cast column-slices of x (contiguous
        # dest) so the xbar transposes read CONTIGUOUS DRAM at full
        # rate. Traced first; order-only fences keep other DMAs out
        # from between the transposes (HW wait budget 1). ----------
        xt = [[None, None] for _ in range(NK)]  # xt[k][h2]: [128c, 1024s]
        last_xp = None
        for h2 in range(2):
            for k in range(NK):
                xc = dram.tile(
                    [1024, 128], BF16, tag=f"xc{h2}_{k}", name=f"xc{h2}_{k}"
                )
                nc.gpsimd.dma_start(
                    xc[:],
                    X[1024 * h2 : 1024 * (h2 + 1), 128 * k : 128 * (k + 1)],
                )
                t = xtp.tile([128, 1024], BF16, tag="xt", bufs=16, name=f"xt{k}_{h2}")
                xt[k][h2] = t
                last_xp = nc.sync.dma_start(t[:], xc[:], transpose=True)
        fence = last_xp

        def fence_dma(instr):
            add_dep_helper(
                instr.ins, fence.ins, sync=False, reason="DMA after transposes"
            )
            return instr

        # ---------- constants ----------
        ones_bf = misc.tile([1, 128], BF16, tag="ones", name="ones_bf")
        nc.gpsimd.memset(ones_bf[:], 1.0)
        ident = misc.tile([128, 128], BF16, tag="ident", name="ident")
        make_identity(nc, ident[:])
        bq_sb = misc.tile([128, 2], F32, tag="bq", name="bq_sb")
        fence_dma(nc.gpsimd.dma_start(bq_sb[:], BQ[:]))
        bk_sb = misc.tile([128, 2], F32, tag="bk", name="bk_sb")
        fence_dma(nc.gpsimd.dma_start(bk_sb[:], BK[:]))
        bv_sb = misc.tile([1, DH], BF16, tag="bv", name="bv_sb")
        fence_dma(nc.gpsimd.dma_start(bv_sb[:], BV[:]))  # SWDGE cast f32->bf16
        bo_sb = misc.tile([1, DH], BF16, tag="bo", name="bo_sb")
        fence_dma(nc.gpsimd.dma_start(bo_sb[:], BO[:]))

        # ---------- weights: one cast DMA per matrix, [128, k, 256] ----------
        def load_w(dram_t, tag):
            t = wp.tile([128, NK, DH], BF16, tag=tag, name=tag)
            fence_dma(
                nc.gpsimd.dma_start(
                    t[:], dram_t[:].rearrange("(k p) n -> p k n", p=128)
                )
            )
            return t

        wq_sb = load_w(WQ, "wq_sb")
        wk_sb = load_w(WK, "wk_sb")
        wv_sb = load_w(WV, "wv_sb")
        wo_sb = load_w(WO, "wo_sb")

        # ---------- stage B: projections ----------
        qt, kt = [], []
        for w_sb, b_sb, dst in ((wq_sb, bq_sb, qt), (wk_sb, bk_sb, kt)):
            for m in range(2):
                slab = qkv.tile([128, S], BF16, tag="qkt", bufs=4, name=f"qkt{m}")
                dst.append(slab)
                for h2 in range(2):
                    ps = ps_big("ps_qk")
                    for half in range(2):
                        for k in range(NK):
                            nc.tensor.matmul(
                                ps[:, 512 * half : 512 * (half + 1)],
                                w_sb[:, k, 128 * m : 128 * (m + 1)],
                                xt[k][h2][:, 512 * half : 512 * (half + 1)],
                                start=(k == 0),
                                stop=(k == NK - 1),
                            )
                    nc.vector.tensor_scalar_add(
                        slab[:, 1024 * h2 : 1024 * (h2 + 1)],
                        ps[:],
                        b_sb[:, m : m + 1],
                    )

        # v natural [s, hd], per-head-65-stride layout with ones column
        vt = []
        for i in range(NST):
            h2, o = divmod(i, 8)
            vtile = qkv.tile([128, 4 * 65], BF16, tag="vt", bufs=NST, name=f"vt{i}")
            vt.append(vtile)
            ps = ps_big("ps_v")
            for k in range(NK):
                nc.tensor.matmul(
                    ps[:, 0:DH],
                    xt[k][h2][:, 128 * o : 128 * (o + 1)],
                    wv_sb[:, k, :],
                    start=(k == 0),
                    stop=False,
                )
            nc.tensor.matmul(
                ps[:, 0:DH], ones_bf[0:1, :], bv_sb[0:1, :], start=False, stop=True
            )
            v_dst = vtile[:].rearrange("p (h x) -> p h x", x=65)[:, :, 0:64]
            v_src = ps[:, 0:DH].rearrange("p (h x) -> p h x", x=64)
            nc.vector.tensor_copy(v_dst, v_src)
            one_cols = vtile[:].rearrange("p (h x) -> p h x", x=65)[:, :, 64:65]
            nc.gpsimd.memset(one_cols, 1.0)

        # ---------- stages C+D, flash-style, software-pipelined AG ----------
        attn = [None] * NST
        pending = []  # deferred D2 emitters

        def emit_d2(g, ag_out):
            atf = atfp.tile([128, NK, 512], BF16, tag="atf", bufs=2, name="atf")
            nc.sync.dma_start(atf[:], ag_out[:].rearrange("(k p) n -> p k n", p=128))
            o_sb = outp.tile([128, 4, DH], F32, tag="o_sb", bufs=2, name="o_sb")
            for ii in range(4):
                po = ps_big("ps_out")
                for k in range(NK):
                    nc.tensor.matmul(
                        po[:, 0:DH],
                        atf[:, k, 128 * ii : 128 * (ii + 1)],
                        wo_sb[:, k, :],
                        start=(k == 0),
                        stop=False,
                    )
                nc.tensor.matmul(
                    po[:, 0:DH], ones_bf[0:1, :], bo_sb[0:1, :],
                    start=False, stop=True,
                )
                nc.vector.tensor_copy(o_sb[:, ii, :], po[:, 0:DH])
            nc.sync.dma_start(
                OUT[512 * g : 512 * (g + 1), :].rearrange("(i p) n -> p i n", p=128),
                o_sb[:],
            )

        for g in range(NSC):
            pv = []
            for ii in range(4):
                t = psum.tile(
                    [128, 4 * 65], F32, tag="ps_pv", bufs=4, name=f"pv{ii}"
                )
                pv.append(t)
            for j in range(4 * g + 4):
                d = j - 4 * g  # >= 0 only on diagonal-overlap blocks
                lo = max(0, 128 * d)
                ptile = ptp.tile([128, 4 * 512], BF16, tag="pt", bufs=3, name="pt")
                for hp in range(2):
                    ps = ps_big("ps_sc")
                    for hh in range(2):
                        nc.tensor.matmul(
                            ps[:, 512 * hh : 512 * (hh + 1)],
                            kt[hp][64 * hh : 64 * hh + 64, 128 * j : 128 * (j + 1)],
                            qt[hp][64 * hh : 64 * hh + 64, 512 * g : 512 * (g + 1)],
                            start=True,
                            stop=True,
                        )
                    # one wide exp for the head pair (masked region is
                    # exp'd too -- bounded junk -- then zeroed below)
                    nc.scalar.activation(
                        ptile[:, 1024 * hp : 1024 * (hp + 1)],
                        ps[:],
                        EXPF,
                        bias=0.0,
                        scale=SCALE,
                    )
                    if d >= 0:
                        for hh in range(2):
                            h = 2 * hp + hh
                            if lo > 0:
                                nc.gpsimd.memset(
                                    ptile[:, 512 * h : 512 * h + lo], 0.0
                                )
                            bnd = ptile[:, 512 * h + lo : 512 * h + lo + 128]
                            nc.gpsimd.affine_select(
                                out=bnd,
                                in_=bnd,
                                compare_op=mybir.AluOpType.is_ge,
                                fill=0.0,
                                base=0,
                                pattern=[[1, 128]],
                                channel_multiplier=-1,
                            )
                # PV: consume this p^T block immediately
                for ii in range(max(0, d), 4):
                    i = 4 * g + ii
                    for h in range(4):
                        nc.tensor.matmul(
                            pv[ii][:, 65 * h : 65 * h + 65],
                            ptile[:, 512 * h + 128 * ii : 512 * h + 128 * ii + 128],
                            vt[j][:].rearrange("p (h x) -> p h x", x=65)[:, h, :],
                            start=(j == 0 and h == 0),
                            stop=(j == i and h == 3),
                        )

            # deferred output projection of the previous chunk (its
            # AllGather has been running during this chunk's compute)
            if pending:
                pending.pop(0)()

            # normalize
            for ii in range(4):
                i = 4 * g + ii
                rl = misc.tile([128, 4], F32, tag="rl", bufs=4, name="rl")
                at = attp.tile([128, DH], BF16, tag="attn", bufs=NST, name=f"at{i}")
                attn[i] = at
                lcols = pv[ii][:].rearrange("p (h x) -> p h x", x=65)[:, :, 64:65]
                nc.vector.reciprocal(rl[:].rearrange("p (h x) -> p h x", x=1), lcols)
                for h in range(4):
                    nc.vector.tensor_scalar_mul(
                        at[:, 64 * h : 64 * h + 64],
                        pv[ii][:, 65 * h : 65 * h + 64],
                        rl[:, h : h + 1],
                    )

            # transpose attn chunk, ship to AllGather
            atTs = attp.tile([128, 2 * 512], BF16, tag="atTs", bufs=2, name="atTs")
            for m in range(2):
                for ii in range(4):
                    i = 4 * g + ii
                    tr = psum.tile(
                        [128, 128], BF16, tag="ps_sc", bufs=2, name="ps_tr"
                    )
                    nc.tensor.transpose(
                        tr[:], attn[i][:, 128 * m : 128 * (m + 1)], ident[:]
                    )
                    nc.vector.tensor_copy(
                        atTs[:, 512 * m + 128 * ii : 512 * m + 128 * (ii + 1)],
                        tr[:],
                    )
            ag_in = dram.tile([DH, 512], BF16, tag=f"ag_in{g}", name=f"ag_in{g}")
            nc.sync.dma_start(
                ag_in[:].rearrange("(m p) n -> p m n", p=128),
                atTs[:].rearrange("p (m n) -> p m n", m=2),
            )
            ag_out = dram.tile([D, 512], BF16, tag=f"ag_out{g}", name=f"ag_out{g}")
            nc.gpsimd.collective_compute(
                "AllGather",
                mybir.AluOpType.bypass,
                replica_groups=REPLICA_GROUPS,
                ins=[ag_in.opt()],
                outs=[ag_out.opt()],
            )
            pending.append(lambda g=g, ago=ag_out: emit_d2(g, ago))

        while pending:
            pending.pop(0)()

    nc.compile()
    return nc


_cached = None


def _get_nc():
    global _cached
    if _cached is None:
        _cached = build_nc()
    return _cached


def make_in_maps(x, Wq, bq, Wk, bk, Wv, bv, Wo, bo):
    in_maps = []
    for c in range(8):
        b, r = divmod(c, TP)
        hsl = slice(r * DH, (r + 1) * DH)
        in_maps.append(
            {
                "x": np.ascontiguousarray(x[b]),
                "wq": np.ascontiguousarray(Wq[:, hsl]),
                "wk": np.ascontiguousarray(Wk[:, hsl]),
                "wv": np.ascontiguousarray(Wv[:, hsl]),
                "wo": np.ascontiguousarray(Wo[:, hsl]),
                "bq2": np.ascontiguousarray(bq[hsl].reshape(2, 128).T),
                "bk2": np.ascontiguousarray(bk[hsl].reshape(2, 128).T),
                "bv_row": np.ascontiguousarray(bv[hsl].reshape(1, DH)),
                "bo_row": np.ascontiguousarray(bo[hsl].reshape(1, DH)),
            }
        )
    return in_maps


def assemble(results):
    """results: list of 8 per-core dicts with 'out' [S, DH] f32."""
    full = np.empty((B, S, D), np.float32)
    for b in range(B):
        full[b] = np.concatenate(
            [np.asarray(results[TP * b + r]["out"]) for r in range(TP)], axis=1
        )
    return full


def run(inputs, **kwargs):
    inputs = {k: np.asarray(v) for k, v in inputs.items()}
    nc = _get_nc()
    in_maps = make_in_maps(**inputs)
    return run_bass_kernel_spmd(nc, in_maps, list(range(8)), **kwargs)


def kernel(**inputs):
    return assemble(run(inputs).results)


# revision 6
# speedup vs baseline: 1.1919x; 1.0773x over previous
"""Multi-head causal attention (B=2, S=2048, D=1024, H=16) on 8 trn2 cores.

Sharding: DP on batch (2 groups of 4 cores), TP on heads within a group
(4 heads/core, column-parallel QKV). The output projection is sharded by
OUTPUT column: per-core attn^T slices are AllGather'd (bf16, on the c
axis) within each 4-core group and each core computes
out[:, r*256:(r+1)*256] = attn_full @ Wo[:, slice] + bo[slice].
Host does pure slicing/concat only.

Per-core pipeline (bf16 matmuls, fp32 PSUM accumulation):
  A. x column-slices cast fp32->bf16 via SWDGE (contiguous-dest), then
     contiguous-source xbar DMA transposes -> x^T SBUF [128c, 1024s].
  B. q^T/k^T = W.T @ x; bias added on DVE (per-partition tensor_scalar),
     bf16. v natural [s, hd] with per-head 65-stride layout + ones
     column (softmax denominator rides the PV matmul).
  C. flash-style per 512-wide sq chunk: scores^T for a head PAIR land
     in one 2-bank PSUM tile (K=64 row-group packing); ONE wide exp per
     pair on ACT (scale=1/8, no max subtraction); causal masking via
     gpsimd memset+affine_select; PV accumulates immediately.
  D. software-pipelined: AllGather of attn^T issued right after each
     chunk; the chunk's output projection is emitted inside the NEXT
     chunk's compute so the collective is fully hidden.
"""

import sys

sys.path.insert(0, "/opt/trn_rl_repo")

from contextlib import ExitStack

import numpy as np

import concourse.bacc as bacc
import concourse.bass as bass
import concourse.tile as tile
from bass_rust import add_dep_helper
from concourse import mybir
from concourse.bass_utils import run_bass_kernel_spmd
from concourse.masks import make_identity

F32 = mybir.dt.float32
BF16 = mybir.dt.bfloat16
EXPF = mybir.ActivationFunctionType.Exp

B, S, D, H, HD = 2, 2048, 1024, 16, 64
TP = 4  # cores per batch group
HPC = H // TP  # 4 local heads
DH = HPC * HD  # 256 local head dims
NK = D // 128  # 8 contraction tiles
NST = S // 128  # 16 s tiles
NSC = S // 512  # 4 s chunks
SCALE = 1.0 / 8.0  # 1/sqrt(HD)

REPLICA_GROUPS = [[0, 1, 2, 3], [4, 5, 6, 7]]


def build_nc():
    nc = bacc.Bacc("TRN2", target_bir_lowering=False, debug=False, num_devices=8)
    X = nc.dram_tensor("x", [S, D], F32, kind="ExternalInput")
    WQ = nc.dram_tensor("wq", [D, DH], F32, kind="ExternalInput")
    WK = nc.dram_tensor("wk", [D, DH], F32, kind="ExternalInput")
    WV = nc.dram_tensor("wv", [D, DH], F32, kind="ExternalInput")
    WO = nc.dram_tensor("wo", [D, DH], F32, kind="ExternalInput")
    BQ = nc.dram_tensor("bq2", [128, 2], F32, kind="ExternalInput")
    BK = nc.dram_tensor("bk2", [128, 2], F32, kind="ExternalInput")
    BV = nc.dram_tensor("bv_row", [1, DH], F32, kind="ExternalInput")
    BO = nc.dram_tensor("bo_row", [1, DH], F32, kind="ExternalInput")
    OUT = nc.dram_tensor("out", [S, DH], F32, kind="ExternalOutput")

    with tile.TileContext(nc) as tc, ExitStack() as ctx:
        xtp = ctx.enter_context(tc.tile_pool(name="xtp", bufs=1))
        ptp = ctx.enter_context(tc.tile_pool(name="ptp", bufs=1))
        qkv = ctx.enter_context(tc.tile_pool(name="qkv", bufs=1))
        wp = ctx.enter_context(tc.tile_pool(name="wp", bufs=1))
        attp = ctx.enter_context(tc.tile_pool(name="attp", bufs=1))
        atfp = ctx.enter_context(tc.tile_pool(name="atfp", bufs=1))
        outp = ctx.enter_context(tc.tile_pool(name="outp", bufs=1))
        misc = ctx.enter_context(tc.tile_pool(name="misc", bufs=1))
        psum = ctx.enter_context(tc.tile_pool(name="psum", bufs=1, space="PSUM"))
        dram = ctx.enter_context(tc.tile_pool(name="dram", bufs=1, space="DRAM"))

        def ps_big(name):  # 2-bank PSUM slots (scores / proj / tr / po)
            return psum.tile([128, 1024], F32, tag="ps_sc", bufs=2, name=name)

        # ---------- stage A: x^T. Cast column-slices of x (contiguous
        # dest) so the xbar transposes read CONTIGUOUS DRAM at full
        # rate. Traced first; order-only fences keep other DMAs out
        # from between the transposes (HW wait budget 1). ----------
        # contiguous cast of x to bf16 (4 row-chunks for lane overlap),
        # then 8 full-column xbar transposes (0.5 MB each)
        x_bf = dram.tile([S, D], BF16, tag="x_bf", name="x_bf")
        for c4 in range(4):
            nc.gpsimd.dma_start(
                x_bf[512 * c4 : 512 * (c4 + 1), :], X[512 * c4 : 512 * (c4 + 1), :]
            )
        xts = []
        last_xp = None
        for k in range(NK):
            t = xtp.tile([128, S], BF16, tag="xt", bufs=NK, name=f"xt{k}")
            xts.append(t)
            last_xp = nc.sync.dma_start(
                t[:], x_bf[:, 128 * k : 128 * (k + 1)], transpose=True
            )
        xt = [[t[:, 0:1024], t[:, 1024:2048]] for t in xts]
        fence = last_xp

        def fence_dma(instr):
            add_dep_helper(
                instr.ins, fence.ins, sync=False, reason="DMA after transposes"
            )
            return instr

        # ---------- constants ----------
        ones_bf = misc.tile([1, 128], BF16, tag="ones", name="ones_bf")
        nc.gpsimd.memset(ones_bf[:], 1.0)
        ident = misc.tile([128, 128], BF16, tag="ident", name="ident")
        make_identity(nc, ident[:])
        bq_sb = misc.tile([128, 2], F32, tag="bq", name="bq_sb")
        fence_dma(nc.gpsimd.dma_start(bq_sb[:], BQ[:]))
        bk_sb = misc.tile([128, 2], F32, tag="bk", name="bk_sb")
        fence_dma(nc.gpsimd.dma_start(bk_sb[:], BK[:]))
        bv_sb = misc.tile([1, DH], BF16, tag="bv", name="bv_sb")
        fence_dma(nc.gpsimd.dma_start(bv_sb[:], BV[:]))  # SWDGE cast f32->bf16
        bo_sb = misc.tile([1, DH], BF16, tag="bo", name="bo_sb")
        fence_dma(nc.gpsimd.dma_start(bo_sb[:], BO[:]))

        # ---------- weights: one cast DMA per matrix, [128, k, 256] ----------
        def load_w(dram_t, tag):
            t = wp.tile([128, NK, DH], BF16, tag=tag, name=tag)
            fence_dma(
                nc.gpsimd.dma_start(
                    t[:], dram_t[:].rearrange("(k p) n -> p k n", p=128)
                )
            )
            return t

        warm_in = dram.tile([128, 1], BF16, tag="warm_in", name="warm_in")
        fence_dma(nc.gpsimd.dma_start(warm_in[:], BQ[:, 0:1]))  # cast, tiny
        warm_out = dram.tile([512, 1], BF16, tag="warm_out", name="warm_out")
        nc.gpsimd.collective_compute(
            "AllGather",
            mybir.AluOpType.bypass,
            replica_groups=REPLICA_GROUPS,
            ins=[warm_in.opt()],
            outs=[warm_out.opt()],
        )

        wq_sb = load_w(WQ, "wq_sb")
        wk_sb = load_w(WK, "wk_sb")
        wv_sb = load_w(WV, "wv_sb")
        wo_sb = load_w(WO, "wo_sb")

        # ---------- stage B: projections ----------
        qt, kt = [], []
        for w_sb, b_sb, dst in ((wq_sb, bq_sb, qt), (wk_sb, bk_sb, kt)):
            for m in range(2):
                slab = qkv.tile([128, S], BF16, tag="qkt", bufs=4, name=f"qkt{m}")
                dst.append(slab)
                for h2 in range(2):
                    ps = ps_big("ps_qk")
                    for half in range(2):
                        for k in range(NK):
                            nc.tensor.matmul(
                                ps[:, 512 * half : 512 * (half + 1)],
                                w_sb[:, k, 128 * m : 128 * (m + 1)],
                                xt[k][h2][:, 512 * half : 512 * (half + 1)],
                                start=(k == 0),
                                stop=(k == NK - 1),
                            )
                    nc.vector.tensor_scalar_add(
                        slab[:, 1024 * h2 : 1024 * (h2 + 1)],
                        ps[:],
                        b_sb[:, m : m + 1],
                    )

        # v natural [s, hd], per-head-65-stride layout with ones column
        vt = []
        for i in range(NST):
            h2, o = divmod(i, 8)
            vtile = qkv.tile([128, 4 * 65], BF16, tag="vt", bufs=NST, name=f"vt{i}")
            vt.append(vtile)
            ps = ps_big("ps_v")
            for k in range(NK):
                nc.tensor.matmul(
                    ps[:, 0:DH],
                    xt[k][h2][:, 128 * o : 128 * (o + 1)],
                    wv_sb[:, k, :],
                    start=(k == 0),
                    stop=False,
                )
            nc.tensor.matmul(
                ps[:, 0:DH], ones_bf[0:1, :], bv_sb[0:1, :], start=False, stop=True
            )
            v_dst = vtile[:].rearrange("p (h x) -> p h x", x=65)[:, :, 0:64]
            v_src = ps[:, 0:DH].rearrange("p (h x) -> p h x", x=64)
            nc.vector.tensor_copy(v_dst, v_src)
            one_cols = vtile[:].rearrange("p (h x) -> p h x", x=65)[:, :, 64:65]
            nc.gpsimd.memset(one_cols, 1.0)

        # ---------- stages C+D, flash-style, software-pipelined AG ----------
        attn = [None] * NST
        pending = []  # deferred D2 emitters

        def emit_d2(g, ag_out):
            atf = atfp.tile([128, NK, 512], BF16, tag="atf", bufs=2, name="atf")
            nc.sync.dma_start(atf[:], ag_out[:].rearrange("(k p) n -> p k n", p=128))
            o_sb = outp.tile([128, 4, DH], F32, tag="o_sb", bufs=2, name="o_sb")
            for ii in range(4):
                po = ps_big("ps_out")
                for k in range(NK):
                    nc.tensor.matmul(
                        po[:, 0:DH],
                        atf[:, k, 128 * ii : 128 * (ii + 1)],
                        wo_sb[:, k, :],
                        start=(k == 0),
                        stop=False,
                    )
                nc.tensor.matmul(
                    po[:, 0:DH], ones_bf[0:1, :], bo_sb[0:1, :],
                    start=False, stop=True,
                )
                nc.vector.tensor_copy(o_sb[:, ii, :], po[:, 0:DH])
            nc.sync.dma_start(
                OUT[512 * g : 512 * (g + 1), :].rearrange("(i p) n -> p i n", p=128),
                o_sb[:],
            )

        for g in range(NSC):
            pv = []
            for ii in range(4):
                t = psum.tile(
                    [128, 4 * 65], F32, tag="ps_pv", bufs=4, name=f"pv{ii}"
                )
                pv.append(t)
            for j in range(4 * g + 4):
                d = j - 4 * g  # >= 0 only on diagonal-overlap blocks
                lo = max(0, 128 * d)
                ptile = ptp.tile([128, 4 * 512], BF16, tag="pt", bufs=3, name="pt")
                for hp in range(2):
                    ps = ps_big("ps_sc")
                    for hh in range(2):
                        nc.tensor.matmul(
                            ps[:, 512 * hh : 512 * (hh + 1)],
                            kt[hp][64 * hh : 64 * hh + 64, 128 * j : 128 * (j + 1)],
                            qt[hp][64 * hh : 64 * hh + 64, 512 * g : 512 * (g + 1)],
                            start=True,
                            stop=True,
                        )
                    # one wide exp for the head pair (masked region is
                    # exp'd too -- bounded junk -- then zeroed below)
                    nc.scalar.activation(
                        ptile[:, 1024 * hp : 1024 * (hp + 1)],
                        ps[:],
                        EXPF,
                        bias=0.0,
                        scale=SCALE,
                    )
                    if d >= 0:
                        for hh in range(2):
                            h = 2 * hp + hh
                            if lo > 0:
                                nc.gpsimd.memset(
                                    ptile[:, 512 * h : 512 * h + lo], 0.0
                                )
                            bnd = ptile[:, 512 * h + lo : 512 * h + lo + 128]
                            nc.gpsimd.affine_select(
                                out=bnd,
                                in_=bnd,
                                compare_op=mybir.AluOpType.is_ge,
                                fill=0.0,
                                base=0,
                                pattern=[[1, 128]],
                                channel_multiplier=-1,
                            )
                # PV: consume this p^T block immediately
                for ii in range(max(0, d), 4):
                    i = 4 * g + ii
                    for h in range(4):
                        nc.tensor.matmul(
                            pv[ii][:, 65 * h : 65 * h + 65],
                            ptile[:, 512 * h + 128 * ii : 512 * h + 128 * ii + 128],
                            vt[j][:].rearrange("p (h x) -> p h x", x=65)[:, h, :],
                            start=(j == 0 and h == 0),
                            stop=(j == i and h == 3),
                        )

            # deferred output projection of the previous chunk (its
            # AllGather has been running during this chunk's compute)
            if pending:
                pending.pop(0)()

            # normalize
            for ii in range(4):
                i = 4 * g + ii
                rl = misc.tile([128, 4], F32, tag="rl", bufs=4, name="rl")
                at = attp.tile([128, DH], BF16, tag="attn", bufs=NST, name=f"at{i}")
                attn[i] = at
                lcols = pv[ii][:].rearrange("p (h x) -> p h x", x=65)[:, :, 64:65]
                nc.vector.reciprocal(rl[:].rearrange("p (h x) -> p h x", x=1), lcols)
                for h in range(4):
                    nc.vector.tensor_scalar_mul(
                        at[:, 64 * h : 64 * h + 64],
                        pv[ii][:, 65 * h : 65 * h + 64],
                        rl[:, h : h + 1],
                    )

            # transpose attn chunk, ship to AllGather
            atTs = attp.tile([128, 2 * 512], BF16, tag="atTs", bufs=2, name="atTs")
            for m in range(2):
                for ii in range(4):
                    i = 4 * g + ii
                    tr = psum.tile(
                        [128, 128], BF16, tag="ps_sc", bufs=2, name="ps_tr"
                    )
                    nc.tensor.transpose(
                        tr[:], attn[i][:, 128 * m : 128 * (m + 1)], ident[:]
                    )
                    nc.vector.tensor_copy(
                        atTs[:, 512 * m + 128 * ii : 512 * m + 128 * (ii + 1)],
                        tr[:],
                    )
            ag_in = dram.tile([DH, 512], BF16, tag=f"ag_in{g}", name=f"ag_in{g}")
            nc.sync.dma_start(
                ag_in[:].rearrange("(m p) n -> p m n", p=128),
                atTs[:].rearrange("p (m n) -> p m n", m=2),
            )
            ag_out = dram.tile([D, 512], BF16, tag=f"ag_out{g}", name=f"ag_out{g}")
            nc.gpsimd.collective_compute(
                "AllGather",
                mybir.AluOpType.bypass,
                replica_groups=REPLICA_GROUPS,
                ins=[ag_in.opt()],
                outs=[ag_out.opt()],
            )
            pending.append(lambda g=g, ago=ag_out: emit_d2(g, ago))

        while pending:
            pending.pop(0)()

    nc.compile()
    return nc


_cached = None


def _get_nc():
    global _cached
    if _cached is None:
        _cached = build_nc()
    return _cached


def make_in_maps(x, Wq, bq, Wk, bk, Wv, bv, Wo, bo):
    in_maps = []
    for c in range(8):
        b, r = divmod(c, TP)
        hsl = slice(r * DH, (r + 1) * DH)
        in_maps.append(
            {
                "x": np.ascontiguousarray(x[b]),
                "wq": np.ascontiguousarray(Wq[:, hsl]),
                "wk": np.ascontiguousarray(Wk[:, hsl]),
                "wv": np.ascontiguousarray(Wv[:, hsl]),
                "wo": np.ascontiguousarray(Wo[:, hsl]),
                "bq2": np.ascontiguousarray(bq[hsl].reshape(2, 128).T),
                "bk2": np.ascontiguousarray(bk[hsl].reshape(2, 128).T),
                "bv_row": np.ascontiguousarray(bv[hsl].reshape(1, DH)),
                "bo_row": np.ascontiguousarray(bo[hsl].reshape(1, DH)),
            }
        )
    return in_maps


def assemble(results):
    """results: list of 8 per-core dicts with 'out' [S, DH] f32."""
    full = np.empty((B, S, D), np.float32)
    for b in range(B):
        full[b] = np.concatenate(
            [np.asarray(results[TP * b + r]["out"]) for r in range(TP)], axis=1
        )
    return full


def run(inputs, **kwargs):
    inputs = {k: np.asarray(v) for k, v in inputs.items()}
    nc = _get_nc()
    in_maps = make_in_maps(**inputs)
    return run_bass_kernel_spmd(nc, in_maps, list(range(8)), **kwargs)


def kernel(**inputs):
    return assemble(run(inputs).results)


# revision 7
# speedup vs baseline: 1.4591x; 1.2242x over previous
"""Multi-head causal attention (B=2, S=2048, D=1024, H=16) on 8 trn2 cores.

Sharding: DP on batch (2 groups of 4 cores), TP on heads within a group
(4 heads/core, column-parallel QKV). The output projection is sharded by
OUTPUT column: per-core attn^T slices are AllGather'd (bf16, on the c
axis) within each 4-core group and each core computes
out[:, r*256:(r+1)*256] = attn_full @ Wo[:, slice] + bo[slice].
Host does pure slicing/concat only.

Per-core pipeline (bf16 matmuls, fp32 PSUM accumulation):
  A. x column-slices cast fp32->bf16 via SWDGE (contiguous-dest), then
     contiguous-source xbar DMA transposes -> x^T SBUF [128c, 1024s].
  B. q^T/k^T = W.T @ x; bias added on DVE (per-partition tensor_scalar),
     bf16. v natural [s, hd] with per-head 65-stride layout + ones
     column (softmax denominator rides the PV matmul).
  C. flash-style per 512-wide sq chunk: scores^T for a head PAIR land
     in one 2-bank PSUM tile (K=64 row-group packing); ONE wide exp per
     pair on ACT (scale=1/8, no max subtraction); causal masking via
     gpsimd memset+affine_select; PV accumulates immediately.
  D. software-pipelined: AllGather of attn^T issued right after each
     chunk; the chunk's output projection is emitted inside the NEXT
     chunk's compute so the collective is fully hidden.
"""

import sys

sys.path.insert(0, "/opt/trn_rl_repo")

from contextlib import ExitStack

import numpy as np

import concourse.bacc as bacc
import concourse.bass as bass
import concourse.tile as tile
from bass_rust import add_dep_helper
from concourse import mybir
from concourse.bass_utils import run_bass_kernel_spmd
from concourse.masks import make_identity

F32 = mybir.dt.float32
BF16 = mybir.dt.bfloat16
EXPF = mybir.ActivationFunctionType.Exp

B, S, D, H, HD = 2, 2048, 1024, 16, 64
TP = 4  # cores per batch group
HPC = H // TP  # 4 local heads
DH = HPC * HD  # 256 local head dims
NK = D // 128  # 8 contraction tiles
NST = S // 128  # 16 s tiles
NSC = S // 512  # 4 s chunks
SCALE = 1.0 / 8.0  # 1/sqrt(HD)

REPLICA_GROUPS = [[0, 1, 2, 3], [4, 5, 6, 7]]


def build_nc():
    nc = bacc.Bacc("TRN2", target_bir_lowering=False, debug=False, num_devices=8)
    X = nc.dram_tensor("x", [S, D], F32, kind="ExternalInput")
    WQ = nc.dram_tensor("wq", [D, DH], F32, kind="ExternalInput")
    WK = nc.dram_tensor("wk", [D, DH], F32, kind="ExternalInput")
    WV = nc.dram_tensor("wv", [D, DH], F32, kind="ExternalInput")
    WO = nc.dram_tensor("wo", [D, DH], F32, kind="ExternalInput")
    BQ = nc.dram_tensor("bq2", [128, 2], F32, kind="ExternalInput")
    BK = nc.dram_tensor("bk2", [128, 2], F32, kind="ExternalInput")
    BV = nc.dram_tensor("bv_row", [1, DH], F32, kind="ExternalInput")
    BO = nc.dram_tensor("bo_row", [1, DH], F32, kind="ExternalInput")
    OUT = nc.dram_tensor("out", [S, DH], F32, kind="ExternalOutput")

    with tile.TileContext(nc) as tc, ExitStack() as ctx:
        xtp = ctx.enter_context(tc.tile_pool(name="xtp", bufs=1))
        ptp = ctx.enter_context(tc.tile_pool(name="ptp", bufs=1))
        qkv = ctx.enter_context(tc.tile_pool(name="qkv", bufs=1))
        wp = ctx.enter_context(tc.tile_pool(name="wp", bufs=1))
        attp = ctx.enter_context(tc.tile_pool(name="attp", bufs=1))
        atfp = ctx.enter_context(tc.tile_pool(name="atfp", bufs=1))
        outp = ctx.enter_context(tc.tile_pool(name="outp", bufs=1))
        misc = ctx.enter_context(tc.tile_pool(name="misc", bufs=1))
        psum = ctx.enter_context(tc.tile_pool(name="psum", bufs=1, space="PSUM"))
        dram = ctx.enter_context(tc.tile_pool(name="dram", bufs=1, space="DRAM"))

        def ps_big(name):  # 2-bank PSUM slots (scores / proj / tr / po)
            return psum.tile([128, 1024], F32, tag="ps_sc", bufs=2, name=name)

        # ---------- constants ----------
        ones_bf = misc.tile([1, 128], BF16, tag="ones", name="ones_bf")
        nc.gpsimd.memset(ones_bf[:], 1.0)
        ident = misc.tile([128, 128], BF16, tag="ident", name="ident")
        make_identity(nc, ident[:])

        # ---------- stage A: x^T via PE transposes. x is cast to bf16
        # during the DMA load (contiguous reads); PE transpose-mode
        # flips [128,128] blocks into PSUM; DVE copies them out in
        # 512-wide batches. PE is busy from ~7us on. ----------
        x_nat = []
        for q4 in range(4):
            xn = xtp.tile([128, 4, 1024], BF16, tag="xnat", bufs=4, name=f"xn{q4}")
            nc.gpsimd.dma_start(
                xn[:],
                X[512 * q4 : 512 * (q4 + 1), :].rearrange("(i p) n -> p i n", p=128),
            )
            x_nat.append(xn)
        xts = []
        for k in range(NK):
            t = xtp.tile([128, S], BF16, tag="xt", bufs=NK, name=f"xt{k}")
            xts.append(t)
            for grp in range(4):
                trp = psum.tile([128, 512], BF16, tag="ps_sc", bufs=2, name="ps_xtr")
                for u in range(4):
                    q4, jj = divmod(4 * grp + u, 4)
                    nc.tensor.transpose(
                        trp[:, 128 * u : 128 * (u + 1)],
                        x_nat[q4][:, jj, 128 * k : 128 * (k + 1)],
                        ident[:],
                    )
                nc.vector.tensor_copy(
                    t[:, 512 * grp : 512 * (grp + 1)], trp[:]
                )
        xt = [[t[:, 0:1024], t[:, 1024:2048]] for t in xts]

        bq_sb = misc.tile([128, 2], F32, tag="bq", name="bq_sb")
        nc.sync.dma_start(bq_sb[:], BQ[:])
        bk_sb = misc.tile([128, 2], F32, tag="bk", name="bk_sb")
        nc.sync.dma_start(bk_sb[:], BK[:])
        bv_sb = misc.tile([1, DH], BF16, tag="bv", name="bv_sb")
        nc.gpsimd.dma_start(bv_sb[:], BV[:])  # SWDGE cast f32->bf16
        bo_sb = misc.tile([1, DH], BF16, tag="bo", name="bo_sb")
        nc.gpsimd.dma_start(bo_sb[:], BO[:])

        # ---------- weights: one cast DMA per matrix, [128, k, 256] ----------
        def load_w(dram_t, tag):
            t = wp.tile([128, NK, DH], BF16, tag=tag, name=tag)
            nc.gpsimd.dma_start(
                t[:], dram_t[:].rearrange("(k p) n -> p k n", p=128)
            )
            return t

        wq_sb = load_w(WQ, "wq_sb")
        wk_sb = load_w(WK, "wk_sb")
        wv_sb = load_w(WV, "wv_sb")
        wo_sb = load_w(WO, "wo_sb")

        # ---------- stage B: projections ----------
        qt, kt = [], []
        for w_sb, b_sb, dst in ((wq_sb, bq_sb, qt), (wk_sb, bk_sb, kt)):
            for m in range(2):
                slab = qkv.tile([128, S], BF16, tag="qkt", bufs=4, name=f"qkt{m}")
                dst.append(slab)
                for h2 in range(2):
                    ps = ps_big("ps_qk")
                    for half in range(2):
                        for k in range(NK):
                            nc.tensor.matmul(
                                ps[:, 512 * half : 512 * (half + 1)],
                                w_sb[:, k, 128 * m : 128 * (m + 1)],
                                xt[k][h2][:, 512 * half : 512 * (half + 1)],
                                start=(k == 0),
                                stop=(k == NK - 1),
                            )
                    nc.vector.tensor_scalar_add(
                        slab[:, 1024 * h2 : 1024 * (h2 + 1)],
                        ps[:],
                        b_sb[:, m : m + 1],
                    )

        # v natural [s, hd], per-head-65-stride layout with ones column
        vt = []
        for i in range(NST):
            h2, o = divmod(i, 8)
            vtile = qkv.tile([128, 4 * 65], BF16, tag="vt", bufs=NST, name=f"vt{i}")
            vt.append(vtile)
            ps = ps_big("ps_v")
            for k in range(NK):
                nc.tensor.matmul(
                    ps[:, 0:DH],
                    xt[k][h2][:, 128 * o : 128 * (o + 1)],
                    wv_sb[:, k, :],
                    start=(k == 0),
                    stop=False,
                )
            nc.tensor.matmul(
                ps[:, 0:DH], ones_bf[0:1, :], bv_sb[0:1, :], start=False, stop=True
            )
            v_dst = vtile[:].rearrange("p (h x) -> p h x", x=65)[:, :, 0:64]
            v_src = ps[:, 0:DH].rearrange("p (h x) -> p h x", x=64)
            nc.vector.tensor_copy(v_dst, v_src)
            one_cols = vtile[:].rearrange("p (h x) -> p h x", x=65)[:, :, 64:65]
            nc.gpsimd.memset(one_cols, 1.0)

        # ---------- stages C+D, flash-style, software-pipelined AG ----------
        attn = [None] * NST
        pending = []  # deferred D2 emitters

        def emit_d2(g, ag_out):
            atf = atfp.tile([128, NK, 512], BF16, tag="atf", bufs=2, name="atf")
            nc.sync.dma_start(atf[:], ag_out[:].rearrange("(k p) n -> p k n", p=128))
            o_sb = outp.tile([128, 4, DH], F32, tag="o_sb", bufs=2, name="o_sb")
            for ii in range(4):
                po = ps_big("ps_out")
                for k in range(NK):
                    nc.tensor.matmul(
                        po[:, 0:DH],
                        atf[:, k, 128 * ii : 128 * (ii + 1)],
                        wo_sb[:, k, :],
                        start=(k == 0),
                        stop=False,
                    )
                nc.tensor.matmul(
                    po[:, 0:DH], ones_bf[0:1, :], bo_sb[0:1, :],
                    start=False, stop=True,
                )
                nc.vector.tensor_copy(o_sb[:, ii, :], po[:, 0:DH])
            nc.sync.dma_start(
                OUT[512 * g : 512 * (g + 1), :].rearrange("(i p) n -> p i n", p=128),
                o_sb[:],
            )

        for g in range(NSC):
            pv = []
            for ii in range(4):
                t = psum.tile(
                    [128, 4 * 65], F32, tag="ps_pv", bufs=4, name=f"pv{ii}"
                )
                pv.append(t)
            for j in range(4 * g + 4):
                d = j - 4 * g  # >= 0 only on diagonal-overlap blocks
                lo = max(0, 128 * d)
                ptile = ptp.tile([128, 4 * 512], BF16, tag="pt", bufs=3, name="pt")
                for hp in range(2):
                    ps = ps_big("ps_sc")
                    for hh in range(2):
                        nc.tensor.matmul(
                            ps[:, 512 * hh : 512 * (hh + 1)],
                            kt[hp][64 * hh : 64 * hh + 64, 128 * j : 128 * (j + 1)],
                            qt[hp][64 * hh : 64 * hh + 64, 512 * g : 512 * (g + 1)],
                            start=True,
                            stop=True,
                        )
                    # one wide exp for the head pair (masked region is
                    # exp'd too -- bounded junk -- then zeroed below)
                    nc.scalar.activation(
                        ptile[:, 1024 * hp : 1024 * (hp + 1)],
                        ps[:],
                        EXPF,
                        bias=0.0,
                        scale=SCALE,
                    )
                    if d >= 0:
                        for hh in range(2):
                            h = 2 * hp + hh
                            if lo > 0:
                                nc.gpsimd.memset(
                                    ptile[:, 512 * h : 512 * h + lo], 0.0
                                )
                            bnd = ptile[:, 512 * h + lo : 512 * h + lo + 128]
                            nc.gpsimd.affine_select(
                                out=bnd,
                                in_=bnd,
                                compare_op=mybir.AluOpType.is_ge,
                                fill=0.0,
                                base=0,
                                pattern=[[1, 128]],
                                channel_multiplier=-1,
                            )
                # PV: consume this p^T block immediately
                for ii in range(max(0, d), 4):
                    i = 4 * g + ii
                    for h in range(4):
                        nc.tensor.matmul(
                            pv[ii][:, 65 * h : 65 * h + 65],
                            ptile[:, 512 * h + 128 * ii : 512 * h + 128 * ii + 128],
                            vt[j][:].rearrange("p (h x) -> p h x", x=65)[:, h, :],
                            start=(j == 0 and h == 0),
                            stop=(j == i and h == 3),
                        )

            # deferred output projection of the previous chunk (its
            # AllGather has been running during this chunk's compute)
            if pending:
                pending.pop(0)()

            # normalize
            for ii in range(4):
                i = 4 * g + ii
                rl = misc.tile([128, 4], F32, tag="rl", bufs=4, name="rl")
                at = attp.tile([128, DH], BF16, tag="attn", bufs=NST, name=f"at{i}")
                attn[i] = at
                lcols = pv[ii][:].rearrange("p (h x) -> p h x", x=65)[:, :, 64:65]
                nc.vector.reciprocal(rl[:].rearrange("p (h x) -> p h x", x=1), lcols)
                for h in range(4):
                    nc.vector.tensor_scalar_mul(
                        at[:, 64 * h : 64 * h + 64],
                        pv[ii][:, 65 * h : 65 * h + 64],
                        rl[:, h : h + 1],
                    )

            # transpose attn chunk, ship to AllGather
            atTs = attp.tile([128, 2 * 512], BF16, tag="atTs", bufs=2, name="atTs")
            for m in range(2):
                for ii in range(4):
                    i = 4 * g + ii
                    tr = psum.tile(
                        [128, 128], BF16, tag="ps_sc", bufs=2, name="ps_tr"
                    )
                    nc.tensor.transpose(
                        tr[:], attn[i][:, 128 * m : 128 * (m + 1)], ident[:]
                    )
                    nc.vector.tensor_copy(
                        atTs[:, 512 * m + 128 * ii : 512 * m + 128 * (ii + 1)],
                        tr[:],
                    )
            ag_in = dram.tile([DH, 512], BF16, tag=f"ag_in{g}", name=f"ag_in{g}")
            nc.sync.dma_start(
                ag_in[:].rearrange("(m p) n -> p m n", p=128),
                atTs[:].rearrange("p (m n) -> p m n", m=2),
            )
            ag_out = dram.tile([D, 512], BF16, tag=f"ag_out{g}", name=f"ag_out{g}")
            nc.gpsimd.collective_compute(
                "AllGather",
                mybir.AluOpType.bypass,
                replica_groups=REPLICA_GROUPS,
                ins=[ag_in.opt()],
                outs=[ag_out.opt()],
            )
            pending.append(lambda g=g, ago=ag_out: emit_d2(g, ago))

        while pending:
            pending.pop(0)()

    nc.compile()
    return nc


_cached = None


def _get_nc():
    global _cached
    if _cached is None:
        _cached = build_nc()
    return _cached


def make_in_maps(x, Wq, bq, Wk, bk, Wv, bv, Wo, bo):
    in_maps = []
    for c in range(8):
        b, r = divmod(c, TP)
        hsl = slice(r * DH, (r + 1) * DH)
        in_maps.append(
            {
                "x": np.ascontiguousarray(x[b]),
                "wq": np.ascontiguousarray(Wq[:, hsl]),
                "wk": np.ascontiguousarray(Wk[:, hsl]),
                "wv": np.ascontiguousarray(Wv[:, hsl]),
                "wo": np.ascontiguousarray(Wo[:, hsl]),
                "bq2": np.ascontiguousarray(bq[hsl].reshape(2, 128).T),
                "bk2": np.ascontiguousarray(bk[hsl].reshape(2, 128).T),
                "bv_row": np.ascontiguousarray(bv[hsl].reshape(1, DH)),
                "bo_row": np.ascontiguousarray(bo[hsl].reshape(1, DH)),
            }
        )
    return in_maps


def assemble(results):
    """results: list of 8 per-core dicts with 'out' [S, DH] f32."""
    full = np.empty((B, S, D), np.float32)
    for b in range(B):
        full[b] = np.concatenate(
            [np.asarray(results[TP * b + r]["out"]) for r in range(TP)], axis=1
        )
    return full


def run(inputs, **kwargs):
    inputs = {k: np.asarray(v) for k, v in inputs.items()}
    nc = _get_nc()
    in_maps = make_in_maps(**inputs)
    return run_bass_kernel_spmd(nc, in_maps, list(range(8)), **kwargs)


def kernel(**inputs):
    return assemble(run(inputs).results)
